# revision 2
# baseline (speedup 1.0000x reference)
"""Trainium2 Bass kernel for nn_ConvThreshold — transposed layout.

conv[p] = sum_{dy,dx in [-2,2]} relu(x)[p+(dy,dx)] * t[p]^(dy^2+dx^2),
t[p] = exp(-1/(2*scale[p]^2)); mask = conv >= 0.5.

Layout: partition = image COLUMN, free = ROW; 8 cores x (image, H-half);
7 overlapping 128-column stripes (124 valid cols each). Per stripe:
ring sums P1,P2,P4,P5,P8 accumulate in PSUM via banded matmuls
(b1/b2/id); [P1|P4] share one 2-bank tile, [P2|P5|P8] one 3-bank tile.
ONE packed ScalarE copy evacuates [P2|P5|P8]; products m1/m3 on Pool,
m2/a2 on DVE; P1/P4 closed by id-matmul accumulation of m1/a2 (emitted
one stripe late = software pipelining); mAB = [g1|g4]*[P1'|P4'] is one
cross-bank DVE op. Weights per stripe: u=s^2 (Pool), 1/u (DVE recip),
t and t^4 (ScalarE exp). relu/mask are DVE tensor_scalar 4x ops.
"""

import sys

sys.path.insert(0, "/opt/trn_rl_repo")

from contextlib import ExitStack

import numpy as np

import concourse.bass as bass
import concourse.tile as tile
from concourse import bacc, mybir
from concourse.bass_utils import run_bass_kernel_spmd

F32 = mybir.dt.float32
F16 = mybir.dt.float16

B, H, W = 4, 768, 768
NCORES = 8
SLAB = H // 2
NST = 7
CW = 124
XR = SLAB + 4          # 388
SEG = XR + SLAB        # 772
ROWS = SLAB            # 384
BANK = 512

_CACHE = {}


def _consts():
    ident = np.eye(128, dtype=np.float16)
    b1 = np.zeros((128, 128), dtype=np.float16)
    b2 = np.zeros((128, 128), dtype=np.float16)
    for m in range(128):
        for d in (-1, 1):
            if 0 <= m + d < 128:
                b1[m + d, m] = 1.0
        for d in (-2, 2):
            if 0 <= m + d < 128:
                b2[m + d, m] = 1.0
    wpack = np.concatenate([b1, b2, ident], axis=1)
    return {"wpack": wpack}


def _build(repeat: int = 1):
    nc = bacc.Bacc(
        "TRN2",
        target_bir_lowering=False,
        debug=False,
        enable_asserts=True,
        num_devices=NCORES,
    )
    ins_d = nc.dram_tensor("ins", [128, NST * SEG], F16, kind="ExternalInput").ap()
    wp_d = nc.dram_tensor("wpack", [128, 384], F16, kind="ExternalInput").ap()
    conv_d = nc.dram_tensor("conv", [128, NST * ROWS], F16, kind="ExternalOutput").ap()
    mask_d = nc.dram_tensor("mask", [128, NST * ROWS], F16, kind="ExternalOutput").ap()

    with tile.TileContext(nc, trace_sim=False) as tc, ExitStack() as ctx:
        sb = ctx.enter_context(tc.tile_pool(name="sb", bufs=3))
        cb = ctx.enter_context(tc.tile_pool(name="cb", bufs=1))
        ps = ctx.enter_context(tc.tile_pool(name="ps", bufs=1, space="PSUM"))

        wp = cb.tile([128, 384], F16, tag="wpack")
        nc.sync.dma_start(wp[:], wp_d[:])
        w_b1, w_b2, w_id = wp[:, 0:128], wp[:, 128:256], wp[:, 256:384]

        def _body():
            ins = sb.tile([128, NST * SEG], F16, tag="ins")
            for k in range(0, NST, 2):
                o, o2 = k * SEG, min(NST, k + 2) * SEG
                nc.sync.dma_start(ins[:, o:o2], ins_d[:, o:o2])
            oc = sb.tile([128, NST * ROWS], F16, tag="oc")
            om = sb.tile([128, NST * ROWS], F16, tag="om")

            st = [dict() for _ in range(NST)]

            def phase1(k):
                o = k * SEG
                xk = ins[:, o : o + XR]
                sk = ins[:, o + XR : o + SEG]
                d = st[k]

                xr = sb.tile([128, XR], F16, tag=f"xr{k%2}")
                nc.vector.tensor_scalar_max(xr[:], xk, 0.0)
                xc = xr[:, 2 : 2 + ROWS]
                xm1, xp1 = xr[:, 1 : 1 + ROWS], xr[:, 3 : 3 + ROWS]
                xm2, xp2 = xr[:, 0:ROWS], xr[:, 4 : 4 + ROWS]
                d["xc"] = xc

                v2 = sb.tile([128, ROWS], F16, tag=f"v2{k%2}")
                nc.vector.tensor_add(v2[:], xm2, xp2)

                u = sb.tile([128, ROWS], F32, tag=f"u{k%2}")
                nc.gpsimd.tensor_mul(u[:], sk, sk)
                vr = sb.tile([128, ROWS], F32, tag=f"vr{k%2}")
                nc.vector.reciprocal_approx_fast(vr[:], u[:])
                g14 = sb.tile([128, 2 * ROWS], F16, tag=f"g14{k%2}")
                d["g14"] = g14
                nc.scalar.activation(
                    g14[:, 0:ROWS], vr[:], mybir.ActivationFunctionType.Exp,
                    scale=-0.5,
                )
                nc.scalar.activation(
                    g14[:, ROWS : 2 * ROWS], vr[:],
                    mybir.ActivationFunctionType.Exp, scale=-2.0,
                )

                pair = ps.tile([128, 2 * BANK], F32, tag=f"pair{k%2}")
                trip = ps.tile([128, 3 * BANK], F32, tag="trip")
                p1 = pair[:, 0:ROWS]
                p4 = pair[:, BANK : BANK + ROWS]
                p2 = trip[:, 0:ROWS]
                p5 = trip[:, BANK : BANK + ROWS]
                p8 = trip[:, 2 * BANK : 2 * BANK + ROWS]
                d.update(pair=pair, trip=trip, p1=p1, p4=p4)

                nc.tensor.matmul(p1, w_b1, xc, start=True, stop=False)
                nc.tensor.matmul(p2, w_b1, xm1, start=True, stop=False)
                nc.tensor.matmul(p2, w_b1, xp1, start=False, stop=True)
                nc.tensor.matmul(p5, w_b1, v2[:], start=True, stop=False)
                nc.tensor.matmul(p4, w_b2, xc, start=True, stop=False)
                nc.tensor.matmul(p5, w_b2, xm1, start=False, stop=False)
                nc.tensor.matmul(p5, w_b2, xp1, start=False, stop=True)
                nc.tensor.matmul(p8, w_b2, v2[:], start=True, stop=True)
                nc.tensor.matmul(p1, w_id, xm1, start=False, stop=False)
                nc.tensor.matmul(p1, w_id, xp1, start=False, stop=False)
                nc.tensor.matmul(p4, w_id, v2[:], start=False, stop=False)

            def phase2(k):
                d = st[k]
                xc, g14 = d["xc"], d["g14"]
                g1, g4 = g14[:, 0:ROWS], g14[:, ROWS : 2 * ROWS]

                # one packed ScalarE evac of [P2|P5|P8]
                e = sb.tile([128, 3 * ROWS], F16, tag=f"e{k%2}")
                nc.scalar.copy(
                    e[:].rearrange("p (b r) -> p b r", b=3, r=ROWS),
                    d["trip"][:].rearrange("p (b r) -> p b r", b=3, r=BANK)[
                        :, :, 0:ROWS
                    ],
                )
                e2, e5, e8 = (
                    e[:, 0:ROWS],
                    e[:, ROWS : 2 * ROWS],
                    e[:, 2 * ROWS : 3 * ROWS],
                )

                m1 = sb.tile([128, ROWS], F16, tag=f"m1{k%2}")
                nc.vector.tensor_mul(m1[:], e2, g1)
                m3 = sb.tile([128, ROWS], F16, tag=f"m3{k%2}")
                nc.vector.tensor_mul(m3[:], e8, g4)
                m2 = sb.tile([128, ROWS], F16, tag=f"m2{k%2}")
                nc.vector.tensor_mul(m2[:], e5, g1)
                a2 = sb.tile([128, ROWS], F16, tag=f"a2{k%2}")
                nc.vector.tensor_add(a2[:], m2[:], m3[:])

                nc.tensor.matmul(d["p1"], w_id, m1[:], start=False, stop=True)
                nc.tensor.matmul(d["p4"], w_id, a2[:], start=False, stop=True)

                mab = sb.tile([128, 2 * ROWS], F16, tag=f"mab{k%2}")
                nc.vector.tensor_mul(
                    mab[:].rearrange("p (b r) -> p b r", b=2, r=ROWS),
                    d["pair"][:].rearrange("p (b r) -> p b r", b=2, r=BANK)[
                        :, :, 0:ROWS
                    ],
                    g14[:].rearrange("p (b r) -> p b r", b=2, r=ROWS),
                )

                co = k * ROWS
                s0 = sb.tile([128, ROWS], F16, tag=f"s0{k%2}")
                nc.vector.tensor_add(s0[:], xc, mab[:, 0:ROWS])
                nc.vector.tensor_add(
                    oc[:, co : co + ROWS], s0[:], mab[:, ROWS : 2 * ROWS]
                )
                nc.vector.tensor_scalar(
                    om[:, co : co + ROWS], oc[:, co : co + ROWS],
                    0.5, None, mybir.AluOpType.is_ge,
                )

            def flush(lo, hi):
                a, b = lo * ROWS, hi * ROWS
                nc.scalar.dma_start(conv_d[:, a:b], oc[:, a:b])
                nc.scalar.dma_start(mask_d[:, a:b], om[:, a:b])

            phase1(0)
            for k in range(NST):
                if k + 1 < NST:
                    phase1(k + 1)
                phase2(k)

            for k in range(0, NST, 2):
                flush(k, min(k + 2, NST))

        if repeat == 1:
            _body()
        else:
            with tc.For_i(0, repeat, 1):
                _body()

    nc.compile()
    return nc


def make_in_maps(bev_map: np.ndarray, bev_scale: np.ndarray):
    consts = _consts()
    in_maps = []
    for c in range(NCORES):
        b, hh = c // 2, c % 2
        xT = np.zeros((124 * NST + 4, 772), dtype=np.float16)
        xT[2:770, 2:770] = bev_map[b, 0].T
        sT = np.ones((124 * NST + 4, 768), dtype=np.float16)
        sT[2:770, :] = bev_scale[b, 0].T
        r0 = hh * SLAB
        ins = np.empty((128, NST * SEG), dtype=np.float16)
        for k in range(NST):
            o = k * SEG
            ins[:, o : o + XR] = xT[124 * k : 124 * k + 128, r0 : r0 + XR]
            ins[:, o + XR : o + SEG] = sT[
                124 * k : 124 * k + 128, r0 : r0 + SLAB
            ]
        m = {"ins": ins}
        m.update({k2: v.copy() for k2, v in consts.items()})
        in_maps.append(m)
    return in_maps


def _unpack(res):
    conv = np.empty((B, 1, H, W), dtype=np.float32)
    mask = np.empty((B, 1, H, W), dtype=np.float32)
    for c in range(NCORES):
        b, hh = c // 2, c % 2
        ocf = np.asarray(res[c]["conv"]).astype(np.float32)
        omf = np.asarray(res[c]["mask"]).astype(np.float32)
        convT = np.empty((W, SLAB), dtype=np.float32)
        maskT = np.empty((W, SLAB), dtype=np.float32)
        for k in range(NST):
            c0 = 124 * k
            n = min(CW, W - c0)
            seg = slice(k * ROWS, (k + 1) * ROWS)
            convT[c0 : c0 + n] = ocf[2 : 2 + n, seg]
            maskT[c0 : c0 + n] = omf[2 : 2 + n, seg]
        r0 = hh * SLAB
        conv[b, 0, r0 : r0 + SLAB, :] = convT.T
        mask[b, 0, r0 : r0 + SLAB, :] = maskT.T
    return conv, mask


def kernel(bev_map: np.ndarray, bev_scale: np.ndarray):
    assert bev_map.shape == (B, 1, H, W) and bev_scale.shape == (B, 1, H, W)
    if "nc" not in _CACHE:
        _CACHE["nc"] = _build()
    nc = _CACHE["nc"]
    in_maps = make_in_maps(bev_map, bev_scale)
    res = run_bass_kernel_spmd(nc, in_maps, list(range(NCORES))).results
    return _unpack(res)


# revision 3
# speedup vs baseline: 1.5461x; 1.5461x over previous
"""Trainium2 Bass kernel for nn_ConvThreshold — transposed layout.

conv[p] = sum_{dy,dx in [-2,2]} relu(x)[p+(dy,dx)] * t[p]^(dy^2+dx^2),
t[p] = exp(-1/(2*scale[p]^2)); mask = conv >= 0.5.

Layout: partition = image COLUMN, free = ROW; 8 cores x (image, H-half);
7 overlapping 128-column stripes (124 valid cols each). Per stripe:
ring sums P1,P2,P4,P5,P8 accumulate in PSUM via banded matmuls
(b1/b2/id); [P1|P4] share one 2-bank tile, [P2|P5|P8] one 3-bank tile.
ONE packed ScalarE copy evacuates [P2|P5|P8]; products m1/m2/m3 and the
combine chain run on DVE (a13 = [P1|P4]+[m1|a2] is one packed cross-bank
add); weights: u=s^2 (ScalarE square), 1/u (DVE fast reciprocal), t and
t^4 (ScalarE exp). relu/mask are DVE tensor_scalar 4x ops. GPSIMD is
left idle on purpose: engines contend for SBUF (~72% aggregate
efficiency when 3 engines run), and its 0.42-efficiency ops cost more
contention than they save.
"""

import sys

sys.path.insert(0, "/opt/trn_rl_repo")

from contextlib import ExitStack

import numpy as np

import concourse.bass as bass
import concourse.tile as tile
from concourse import bacc, mybir
from concourse.bass_utils import run_bass_kernel_spmd

F32 = mybir.dt.float32
F16 = mybir.dt.float16

B, H, W = 4, 768, 768
NCORES = 8
SLAB = H // 2
NST = 7
CW = 124
XR = SLAB + 4          # 388
SEG = XR + SLAB        # 772
ROWS = SLAB            # 384
BANK = 512

_CACHE = {}


def _consts():
    ident = np.eye(128, dtype=np.float16)
    b1 = np.zeros((128, 128), dtype=np.float16)
    b2 = np.zeros((128, 128), dtype=np.float16)
    for m in range(128):
        for d in (-1, 1):
            if 0 <= m + d < 128:
                b1[m + d, m] = 1.0
        for d in (-2, 2):
            if 0 <= m + d < 128:
                b2[m + d, m] = 1.0
    wpack = np.concatenate([b1, b2, ident], axis=1)
    return {"wpack": wpack}


def _build(repeat: int = 1):
    nc = bacc.Bacc(
        "TRN2",
        target_bir_lowering=False,
        debug=False,
        enable_asserts=True,
        num_devices=NCORES,
    )
    ins_d = nc.dram_tensor("ins", [128, NST * SEG], F16, kind="ExternalInput").ap()
    wp_d = nc.dram_tensor("wpack", [128, 384], F16, kind="ExternalInput").ap()
    conv_d = nc.dram_tensor("conv", [128, NST * ROWS], F16, kind="ExternalOutput").ap()
    mask_d = nc.dram_tensor("mask", [128, NST * ROWS], F16, kind="ExternalOutput").ap()

    with tile.TileContext(nc, trace_sim=False) as tc, ExitStack() as ctx:
        sb = ctx.enter_context(tc.tile_pool(name="sb", bufs=3))
        cb = ctx.enter_context(tc.tile_pool(name="cb", bufs=1))
        ps = ctx.enter_context(tc.tile_pool(name="ps", bufs=1, space="PSUM"))

        wp = cb.tile([128, 384], F16, tag="wpack")
        nc.sync.dma_start(wp[:], wp_d[:])
        w_b1, w_b2, w_id = wp[:, 0:128], wp[:, 128:256], wp[:, 256:384]

        def _body():
            ins = sb.tile([128, NST * SEG], F16, tag="ins")
            for k in range(0, NST, 2):
                o, o2 = k * SEG, min(NST, k + 2) * SEG
                nc.sync.dma_start(ins[:, o:o2], ins_d[:, o:o2])
            oc = sb.tile([128, NST * ROWS], F16, tag="oc")
            om = sb.tile([128, NST * ROWS], F16, tag="om")

            st = [dict() for _ in range(NST)]

            def phase1(k):
                o = k * SEG
                xk = ins[:, o : o + XR]
                sk = ins[:, o + XR : o + SEG]
                d = st[k]

                xr = sb.tile([128, XR], F16, tag=f"xr{k%2}")
                nc.vector.tensor_scalar_max(xr[:], xk, 0.0)
                xc = xr[:, 2 : 2 + ROWS]
                xm1, xp1 = xr[:, 1 : 1 + ROWS], xr[:, 3 : 3 + ROWS]
                xm2, xp2 = xr[:, 0:ROWS], xr[:, 4 : 4 + ROWS]
                d["xc"] = xc

                v2 = sb.tile([128, ROWS], F16, tag=f"v2{k%2}")
                nc.vector.tensor_add(v2[:], xm2, xp2)

                u = sb.tile([128, ROWS], F32, tag=f"u{k%2}")
                nc.scalar.activation(
                    u[:], sk, mybir.ActivationFunctionType.Square
                )
                vr = sb.tile([128, ROWS], F32, tag=f"vr{k%2}")
                nc.vector.reciprocal_approx_fast(vr[:], u[:])
                g14 = sb.tile([128, 2 * ROWS], F16, tag=f"g14{k%2}")
                d["g14"] = g14
                nc.scalar.activation(
                    g14[:, 0:ROWS], vr[:], mybir.ActivationFunctionType.Exp,
                    scale=-0.5,
                )
                nc.scalar.activation(
                    g14[:, ROWS : 2 * ROWS], vr[:],
                    mybir.ActivationFunctionType.Exp, scale=-2.0,
                )

                pair = ps.tile([128, 2 * BANK], F32, tag=f"pair{k%2}")
                trip = ps.tile([128, 3 * BANK], F32, tag="trip")
                p1 = pair[:, 0:ROWS]
                p4 = pair[:, BANK : BANK + ROWS]
                p2 = trip[:, 0:ROWS]
                p5 = trip[:, BANK : BANK + ROWS]
                p8 = trip[:, 2 * BANK : 2 * BANK + ROWS]
                d.update(pair=pair, trip=trip, p1=p1, p4=p4)

                nc.tensor.matmul(p1, w_b1, xc, start=True, stop=False)
                nc.tensor.matmul(p2, w_b1, xm1, start=True, stop=False)
                nc.tensor.matmul(p2, w_b1, xp1, start=False, stop=True)
                nc.tensor.matmul(p5, w_b1, v2[:], start=True, stop=False)
                nc.tensor.matmul(p4, w_b2, xc, start=True, stop=False)
                nc.tensor.matmul(p5, w_b2, xm1, start=False, stop=False)
                nc.tensor.matmul(p5, w_b2, xp1, start=False, stop=True)
                nc.tensor.matmul(p8, w_b2, v2[:], start=True, stop=True)
                nc.tensor.matmul(p1, w_id, xm1, start=False, stop=False)
                nc.tensor.matmul(p1, w_id, xp1, start=False, stop=False)
                nc.tensor.matmul(p4, w_id, v2[:], start=False, stop=False)

            def phase2(k):
                d = st[k]
                xc, g14 = d["xc"], d["g14"]
                g1, g4 = g14[:, 0:ROWS], g14[:, ROWS : 2 * ROWS]

                # one packed ScalarE evac of [P2|P5|P8]
                e = sb.tile([128, 3 * ROWS], F16, tag=f"e{k%2}")
                nc.scalar.copy(
                    e[:].rearrange("p (b r) -> p b r", b=3, r=ROWS),
                    d["trip"][:].rearrange("p (b r) -> p b r", b=3, r=BANK)[
                        :, :, 0:ROWS
                    ],
                )
                e2, e5, e8 = (
                    e[:, 0:ROWS],
                    e[:, ROWS : 2 * ROWS],
                    e[:, 2 * ROWS : 3 * ROWS],
                )

                m1 = sb.tile([128, ROWS], F16, tag=f"m1{k%2}")
                nc.vector.tensor_mul(m1[:], e2, g1)
                m3 = sb.tile([128, ROWS], F16, tag=f"m3{k%2}")
                nc.vector.tensor_mul(m3[:], e8, g4)
                m2 = sb.tile([128, ROWS], F16, tag=f"m2{k%2}")
                nc.vector.tensor_mul(m2[:], e5, g1)
                a2 = sb.tile([128, ROWS], F16, tag=f"a2{k%2}")
                nc.vector.tensor_add(a2[:], m2[:], m3[:])

                nc.tensor.matmul(d["p1"], w_id, m1[:], start=False, stop=True)
                nc.tensor.matmul(d["p4"], w_id, a2[:], start=False, stop=True)

                mab = sb.tile([128, 2 * ROWS], F16, tag=f"mab{k%2}")
                nc.vector.tensor_mul(
                    mab[:].rearrange("p (b r) -> p b r", b=2, r=ROWS),
                    d["pair"][:].rearrange("p (b r) -> p b r", b=2, r=BANK)[
                        :, :, 0:ROWS
                    ],
                    g14[:].rearrange("p (b r) -> p b r", b=2, r=ROWS),
                )

                co = k * ROWS
                s0 = sb.tile([128, ROWS], F16, tag=f"s0{k%2}")
                nc.vector.tensor_add(s0[:], xc, mab[:, 0:ROWS])
                nc.vector.tensor_add(
                    oc[:, co : co + ROWS], s0[:], mab[:, ROWS : 2 * ROWS]
                )
                nc.vector.tensor_scalar(
                    om[:, co : co + ROWS], oc[:, co : co + ROWS],
                    0.5, None, mybir.AluOpType.is_ge,
                )

            def flush(lo, hi):
                a, b = lo * ROWS, hi * ROWS
                nc.scalar.dma_start(conv_d[:, a:b], oc[:, a:b])
                nc.scalar.dma_start(mask_d[:, a:b], om[:, a:b])

            phase1(0)
            for k in range(NST):
                if k + 1 < NST:
                    phase1(k + 1)
                phase2(k)

            for k in range(0, NST, 2):
                flush(k, min(k + 2, NST))

        if repeat == 1:
            _body()
        else:
            with tc.For_i(0, repeat, 1):
                _body()

    nc.compile()
    return nc


def make_in_maps(bev_map: np.ndarray, bev_scale: np.ndarray):
    consts = _consts()
    in_maps = []
    for c in range(NCORES):
        b, hh = c // 2, c % 2
        xT = np.zeros((124 * NST + 4, 772), dtype=np.float16)
        xT[2:770, 2:770] = bev_map[b, 0].T
        sT = np.ones((124 * NST + 4, 768), dtype=np.float16)
        sT[2:770, :] = bev_scale[b, 0].T
        r0 = hh * SLAB
        ins = np.empty((128, NST * SEG), dtype=np.float16)
        for k in range(NST):
            o = k * SEG
            ins[:, o : o + XR] = xT[124 * k : 124 * k + 128, r0 : r0 + XR]
            ins[:, o + XR : o + SEG] = sT[
                124 * k : 124 * k + 128, r0 : r0 + SLAB
            ]
        m = {"ins": ins}
        m.update({k2: v.copy() for k2, v in consts.items()})
        in_maps.append(m)
    return in_maps


def _unpack(res):
    conv = np.empty((B, 1, H, W), dtype=np.float32)
    mask = np.empty((B, 1, H, W), dtype=np.float32)
    for c in range(NCORES):
        b, hh = c // 2, c % 2
        ocf = np.asarray(res[c]["conv"]).astype(np.float32)
        omf = np.asarray(res[c]["mask"]).astype(np.float32)
        convT = np.empty((W, SLAB), dtype=np.float32)
        maskT = np.empty((W, SLAB), dtype=np.float32)
        for k in range(NST):
            c0 = 124 * k
            n = min(CW, W - c0)
            seg = slice(k * ROWS, (k + 1) * ROWS)
            convT[c0 : c0 + n] = ocf[2 : 2 + n, seg]
            maskT[c0 : c0 + n] = omf[2 : 2 + n, seg]
        r0 = hh * SLAB
        conv[b, 0, r0 : r0 + SLAB, :] = convT.T
        mask[b, 0, r0 : r0 + SLAB, :] = maskT.T
    return conv, mask


def kernel(bev_map: np.ndarray, bev_scale: np.ndarray):
    assert bev_map.shape == (B, 1, H, W) and bev_scale.shape == (B, 1, H, W)
    if "nc" not in _CACHE:
        _CACHE["nc"] = _build()
    nc = _CACHE["nc"]
    in_maps = make_in_maps(bev_map, bev_scale)
    res = run_bass_kernel_spmd(nc, in_maps, list(range(NCORES))).results
    return _unpack(res)


# revision 4
# speedup vs baseline: 1.6121x; 1.0427x over previous
"""Trainium2 Bass kernel for nn_ConvThreshold — transposed layout, V6: products consolidated on DVE.

conv[p] = sum_{dy,dx in [-2,2]} relu(x)[p+(dy,dx)] * t[p]^(dy^2+dx^2),
t[p] = exp(-1/(2*scale[p]^2)); mask = conv >= 0.5.

Layout: partition = image COLUMN, free = ROW; 8 cores x (image, H-half);
7 overlapping 128-column stripes (124 valid cols each). Per stripe:
ring sums P1,P2,P4,P5,P8 accumulate in PSUM via banded matmuls
(b1/b2/id); [P1|P4] share one 2-bank tile, [P2|P5|P8] one 3-bank tile.
ONE packed ScalarE copy evacuates [P2|P5|P8]; products m1/m3 on Pool,
m2/a2 on DVE; P1/P4 closed by id-matmul accumulation of m1/a2 (emitted
one stripe late = software pipelining); mAB = [g1|g4]*[P1'|P4'] is one
cross-bank DVE op. Weights per stripe: u=s^2 (Pool), 1/u (DVE recip),
t and t^4 (ScalarE exp). relu/mask are DVE tensor_scalar 4x ops.
"""

import sys

sys.path.insert(0, "/opt/trn_rl_repo")

from contextlib import ExitStack

import numpy as np

import concourse.bass as bass
import concourse.tile as tile
from concourse import bacc, mybir
from concourse.bass_utils import run_bass_kernel_spmd

F32 = mybir.dt.float32
F16 = mybir.dt.float16

B, H, W = 4, 768, 768
NCORES = 8
SLAB = H // 2
NST = 7
CW = 124
XR = SLAB + 4          # 388
SEG = XR + SLAB        # 772
ROWS = SLAB            # 384
BANK = 512

_CACHE = {}


def _consts():
    ident = np.eye(128, dtype=np.float16)
    b1 = np.zeros((128, 128), dtype=np.float16)
    b2 = np.zeros((128, 128), dtype=np.float16)
    for m in range(128):
        for d in (-1, 1):
            if 0 <= m + d < 128:
                b1[m + d, m] = 1.0
        for d in (-2, 2):
            if 0 <= m + d < 128:
                b2[m + d, m] = 1.0
    wpack = np.concatenate([b1, b2, ident], axis=1)
    return {"wpack": wpack}


def _build(repeat: int = 1):
    nc = bacc.Bacc(
        "TRN2",
        target_bir_lowering=False,
        debug=False,
        enable_asserts=True,
        num_devices=NCORES,
    )
    ins_d = nc.dram_tensor("ins", [128, NST * SEG], F16, kind="ExternalInput").ap()
    wp_d = nc.dram_tensor("wpack", [128, 384], F16, kind="ExternalInput").ap()
    conv_d = nc.dram_tensor("conv", [128, NST * ROWS], F16, kind="ExternalOutput").ap()
    mask_d = nc.dram_tensor("mask", [128, NST * ROWS], F16, kind="ExternalOutput").ap()

    with tile.TileContext(nc, trace_sim=False) as tc, ExitStack() as ctx:
        sb = ctx.enter_context(tc.tile_pool(name="sb", bufs=3))
        cb = ctx.enter_context(tc.tile_pool(name="cb", bufs=1))
        ps = ctx.enter_context(tc.tile_pool(name="ps", bufs=1, space="PSUM"))

        wp = cb.tile([128, 384], F16, tag="wpack")
        nc.sync.dma_start(wp[:], wp_d[:])
        w_b1, w_b2, w_id = wp[:, 0:128], wp[:, 128:256], wp[:, 256:384]

        def _body():
            ins = sb.tile([128, NST * SEG], F16, tag="ins")
            for k in range(0, NST, 2):
                o, o2 = k * SEG, min(NST, k + 2) * SEG
                nc.sync.dma_start(ins[:, o:o2], ins_d[:, o:o2])
            oc = sb.tile([128, NST * ROWS], F16, tag="oc")
            om = sb.tile([128, NST * ROWS], F16, tag="om")

            st = [dict() for _ in range(NST)]

            def phase1(k):
                o = k * SEG
                xk = ins[:, o : o + XR]
                sk = ins[:, o + XR : o + SEG]
                d = st[k]

                xr = sb.tile([128, XR], F16, tag=f"xr{k%2}")
                nc.vector.tensor_scalar_max(xr[:], xk, 0.0)
                xc = xr[:, 2 : 2 + ROWS]
                xm1, xp1 = xr[:, 1 : 1 + ROWS], xr[:, 3 : 3 + ROWS]
                xm2, xp2 = xr[:, 0:ROWS], xr[:, 4 : 4 + ROWS]
                d["xc"] = xc

                u = sb.tile([128, ROWS], F32, tag=f"u{k%2}")
                nc.scalar.activation(
                    u[:], sk, mybir.ActivationFunctionType.Square
                )
                vr = sb.tile([128, ROWS], F32, tag=f"vr{k%2}")
                nc.vector.reciprocal_approx_fast(vr[:], u[:])
                g14 = sb.tile([128, 2 * ROWS], F16, tag=f"g14{k%2}")
                d["g14"] = g14
                nc.scalar.activation(
                    g14[:, 0:ROWS], vr[:], mybir.ActivationFunctionType.Exp,
                    scale=-0.5,
                )
                nc.scalar.activation(
                    g14[:, ROWS : 2 * ROWS], vr[:],
                    mybir.ActivationFunctionType.Exp, scale=-2.0,
                )

                pair = ps.tile([128, 2 * BANK], F32, tag=f"pair{k%2}")
                trip = ps.tile([128, 3 * BANK], F32, tag="trip")
                p1 = pair[:, 0:ROWS]
                p4 = pair[:, BANK : BANK + ROWS]
                p2 = trip[:, 0:ROWS]
                p5 = trip[:, BANK : BANK + ROWS]
                p8 = trip[:, 2 * BANK : 2 * BANK + ROWS]
                d.update(pair=pair, trip=trip, p1=p1, p4=p4)

                nc.tensor.matmul(p1, w_b1, xc, start=True, stop=False)
                nc.tensor.matmul(p2, w_b1, xm1, start=True, stop=False)
                nc.tensor.matmul(p2, w_b1, xp1, start=False, stop=True)
                nc.tensor.matmul(p5, w_b1, xm2, start=True, stop=False)
                nc.tensor.matmul(p5, w_b1, xp2, start=False, stop=False)
                nc.tensor.matmul(p4, w_b2, xc, start=True, stop=False)
                nc.tensor.matmul(p5, w_b2, xm1, start=False, stop=False)
                nc.tensor.matmul(p5, w_b2, xp1, start=False, stop=True)
                nc.tensor.matmul(p8, w_b2, xm2, start=True, stop=False)
                nc.tensor.matmul(p8, w_b2, xp2, start=False, stop=True)
                nc.tensor.matmul(p1, w_id, xm1, start=False, stop=False)
                nc.tensor.matmul(p1, w_id, xp1, start=False, stop=False)
                nc.tensor.matmul(p4, w_id, xm2, start=False, stop=False)
                nc.tensor.matmul(p4, w_id, xp2, start=False, stop=False)

            def phase2(k):
                d = st[k]
                xc, g14 = d["xc"], d["g14"]
                g1, g4 = g14[:, 0:ROWS], g14[:, ROWS : 2 * ROWS]

                # one packed ScalarE evac of [P2|P5|P8]
                e = sb.tile([128, 3 * ROWS], F16, tag=f"e{k%2}")
                nc.scalar.copy(
                    e[:].rearrange("p (b r) -> p b r", b=3, r=ROWS),
                    d["trip"][:].rearrange("p (b r) -> p b r", b=3, r=BANK)[
                        :, :, 0:ROWS
                    ],
                )
                e2, e5, e8 = (
                    e[:, 0:ROWS],
                    e[:, ROWS : 2 * ROWS],
                    e[:, 2 * ROWS : 3 * ROWS],
                )

                m1 = sb.tile([128, ROWS], F16, tag=f"m1{k%2}")
                nc.vector.tensor_mul(m1[:], e2, g1)
                m3 = sb.tile([128, ROWS], F16, tag=f"m3{k%2}")
                nc.vector.tensor_mul(m3[:], e8, g4)
                m2 = sb.tile([128, ROWS], F16, tag=f"m2{k%2}")
                nc.vector.tensor_mul(m2[:], e5, g1)
                a2 = sb.tile([128, ROWS], F16, tag=f"a2{k%2}")
                nc.vector.tensor_add(a2[:], m2[:], m3[:])

                nc.tensor.matmul(d["p1"], w_id, m1[:], start=False, stop=True)
                nc.tensor.matmul(d["p4"], w_id, a2[:], start=False, stop=True)

                mab = sb.tile([128, 2 * ROWS], F16, tag=f"mab{k%2}")
                nc.vector.tensor_mul(
                    mab[:].rearrange("p (b r) -> p b r", b=2, r=ROWS),
                    d["pair"][:].rearrange("p (b r) -> p b r", b=2, r=BANK)[
                        :, :, 0:ROWS
                    ],
                    g14[:].rearrange("p (b r) -> p b r", b=2, r=ROWS),
                )

                co = k * ROWS
                s0 = sb.tile([128, ROWS], F16, tag=f"s0{k%2}")
                nc.vector.tensor_add(s0[:], xc, mab[:, 0:ROWS])
                nc.vector.tensor_add(
                    oc[:, co : co + ROWS], s0[:], mab[:, ROWS : 2 * ROWS]
                )
                nc.vector.tensor_scalar(
                    om[:, co : co + ROWS], oc[:, co : co + ROWS],
                    0.5, None, mybir.AluOpType.is_ge,
                )

            def flush(lo, hi):
                a, b = lo * ROWS, hi * ROWS
                nc.scalar.dma_start(conv_d[:, a:b], oc[:, a:b])
                nc.scalar.dma_start(mask_d[:, a:b], om[:, a:b])

            phase1(0)
            for k in range(NST):
                if k + 1 < NST:
                    phase1(k + 1)
                phase2(k)

            for k in range(0, NST, 2):
                flush(k, min(k + 2, NST))

        if repeat == 1:
            _body()
        elif repeat % 2 == 0:
            with tc.For_i(0, repeat // 2, 1):
                _body()
                _body()
        else:
            with tc.For_i(0, repeat, 1):
                _body()

    nc.compile()
    return nc


def make_in_maps(bev_map: np.ndarray, bev_scale: np.ndarray):
    consts = _consts()
    in_maps = []
    for c in range(NCORES):
        b, hh = c // 2, c % 2
        xT = np.zeros((124 * NST + 4, 772), dtype=np.float16)
        xT[2:770, 2:770] = bev_map[b, 0].T
        sT = np.ones((124 * NST + 4, 768), dtype=np.float16)
        sT[2:770, :] = bev_scale[b, 0].T
        r0 = hh * SLAB
        ins = np.empty((128, NST * SEG), dtype=np.float16)
        for k in range(NST):
            o = k * SEG
            ins[:, o : o + XR] = xT[124 * k : 124 * k + 128, r0 : r0 + XR]
            ins[:, o + XR : o + SEG] = sT[
                124 * k : 124 * k + 128, r0 : r0 + SLAB
            ]
        m = {"ins": ins}
        m.update({k2: v.copy() for k2, v in consts.items()})
        in_maps.append(m)
    return in_maps


def _unpack(res):
    conv = np.empty((B, 1, H, W), dtype=np.float32)
    mask = np.empty((B, 1, H, W), dtype=np.float32)
    for c in range(NCORES):
        b, hh = c // 2, c % 2
        ocf = np.asarray(res[c]["conv"]).astype(np.float32)
        omf = np.asarray(res[c]["mask"]).astype(np.float32)
        convT = np.empty((W, SLAB), dtype=np.float32)
        maskT = np.empty((W, SLAB), dtype=np.float32)
        for k in range(NST):
            c0 = 124 * k
            n = min(CW, W - c0)
            seg = slice(k * ROWS, (k + 1) * ROWS)
            convT[c0 : c0 + n] = ocf[2 : 2 + n, seg]
            maskT[c0 : c0 + n] = omf[2 : 2 + n, seg]
        r0 = hh * SLAB
        conv[b, 0, r0 : r0 + SLAB, :] = convT.T
        mask[b, 0, r0 : r0 + SLAB, :] = maskT.T
    return conv, mask


def kernel(bev_map: np.ndarray, bev_scale: np.ndarray):
    assert bev_map.shape == (B, 1, H, W) and bev_scale.shape == (B, 1, H, W)
    if "nc" not in _CACHE:
        _CACHE["nc"] = _build()
    nc = _CACHE["nc"]
    in_maps = make_in_maps(bev_map, bev_scale)
    res = run_bass_kernel_spmd(nc, in_maps, list(range(NCORES))).results
    return _unpack(res)
